# revision 49
# baseline (speedup 1.0000x reference)
"""MADPSNet MoE-routing kernel for 8 Trainium2 NeuronCores.

The reference computes every expert on the full stacked input and then
gathers one expert per agent.  The routing indices (laac_shallow /
laac_deep) are host-visible numpy values, so we do the routing on the
host: per agent we select the 4 weight matrices of its chosen experts
and run only the selected chain

    x[2048,256] @ W1[256,512] -> relu -> @ W2[512,256] -> relu
                -> @ W3[256,512] -> relu -> @ W4[512,128] (+bias)

One agent per NeuronCore (A == 8 == n_cores), no collectives.

Layout: everything feature-major on chip (features on the 128
partitions, batch on the free dim).  The host pre-packs

    x   [128, 4096]     col = bt*1024 + k*512 + b  (bt-major batch tiles)
    wN  [128, K/128*M]  col = (k*mc + m)*128 + j   (k-chunk-major)
    bias[128, 11]       col j = 128-chunk j of [b1(4) b2(2) b3(4) b4(1)]

All tensors stream as bf16 (accumulate fp32 in PSUM; the 2e-2 rel-err
budget leaves ~5x headroom), which halves HBM traffic and lets
LDWEIGHTS use FWL.  Every DMA is a large contiguous transfer, issued
in compute-need order on the two HWDGE queues (x on sync/SP, weights
on scalar/ACT).  Matmuls accumulate fp32 in PSUM, bias+relu runs split
across ScalarE and VectorE with a fixed engine per destination tile,
and the layers are emitted as a (bt + 2*layer) diagonal wavefront so
the in-order PE queue always has ready work while L1 waits on x DMAs.
A few warm-up matmuls on an uninitialized scratch tile (no deps, so
they issue the moment the PE queue opens) keep the PE busy from kernel
start so the HAM clock un-throttles (1.2 -> 2.4 GHz) before the real
work arrives.  The kernel returns out^T [128, 2048] bf16 per core; the
host transposes back and upcasts.
"""

import os

import numpy as np

import concourse.bass as bass
import concourse.mybir as mybir
from concourse import bacc
from concourse.bass_utils import run_bass_kernel_spmd
from concourse.tile import TileContext

A, B, S = 8, 2048, 256
H1, H2, D1, D2 = 512, 256, 512, 128
P = 128
BT = 512            # batch tile (psum bank: 512 fp32)
NBT = B // BT

_DT_MAP = {
    "f32": mybir.dt.float32,
    "f32r": mybir.dt.float32r,
    "bf16": mybir.dt.bfloat16,
}

# layer: (k_chunks, m_chunks, bias col offset, relu?)
_LAYERS = [
    (S // P, H1 // P, 0, True),    # L1: 256 -> 512
    (H1 // P, H2 // P, 4, True),   # L2: 512 -> 256
    (H2 // P, D1 // P, 6, True),   # L3: 256 -> 512
    (D1 // P, D2 // P, 10, False), # L4: 512 -> 128
]


def _build(
    dt_name: str,
    add_bias: bool,
    warm: int,
    junk: int,
    pre: int = 1,
    jpost: int = 4,
) -> bass.Bass:
    dt = _DT_MAP[dt_name]
    f32 = mybir.dt.float32
    nc = bacc.Bacc(None, target_bir_lowering=False, debug=False)

    x_d = nc.dram_tensor("x", [P, (S // P) * B], dt, kind="ExternalInput")
    w_ds = [
        nc.dram_tensor("w1", [P, (S // P) * H1], dt, kind="ExternalInput"),
        nc.dram_tensor("w2", [P, (H1 // P) * H2], dt, kind="ExternalInput"),
        nc.dram_tensor("w3", [P, (H2 // P) * D1], dt, kind="ExternalInput"),
        nc.dram_tensor("w4", [P, (D1 // P) * D2], dt, kind="ExternalInput"),
    ]
    b_d = (
        nc.dram_tensor("bias", [P, 11], f32, kind="ExternalInput")
        if add_bias
        else None
    )
    out_dt = dt if dt == mybir.dt.bfloat16 else f32
    out_d = nc.dram_tensor("out", [D2, B], out_dt, kind="ExternalOutput")

    # Raw (non-tile) scratch for warm-up / clock-hold matmuls: SBUF
    # contents are garbage (never read downstream; the PSUM sink is
    # never read at all), so no memset and no dependencies anywhere.
    # Raw tensors let these matmuls run OUTSIDE the TileContext:
    #  - `pre` matmuls before the tile entry barrier start the HAM clock
    #    ramp ~1.1us earlier (the PE reaches the barrier last, delaying
    #    the input DMAs ~0.2us -- net win for one matmul);
    #  - `jpost` matmuls after the tile exit keep the PE busy into the
    #    framework's semaphore-zeroing teardown so it runs at full clock
    #    instead of the 4/8 idle-gated rate.
    import contextlib

    _raw = contextlib.ExitStack()
    wdt = f32 if dt == mybir.dt.float32r else dt
    wsb = _raw.enter_context(nc.sbuf_tensor([P, BT], wdt))
    wps = _raw.enter_context(nc.psum_tensor([P, BT], f32))
    wlhs = wsb[:, 0:P]
    wrhs = wsb[:]
    if dt == mybir.dt.float32r:
        wlhs = wlhs.bitcast(dt)
        wrhs = wrhs.bitcast(dt)
    for _ in range(pre):
        nc.tensor.matmul(wps[:], wlhs, wrhs, start=True, stop=True)

    with TileContext(nc) as tc:
        with (
            tc.tile_pool(name="persist", bufs=1) as pp,
            tc.tile_pool(name="psum", bufs=7, space="PSUM") as psp,
        ):
            xt = pp.tile([P, (S // P) * B], dt, tag="xt", name="xt")
            wts = [
                pp.tile(
                    [P, w_ds[i].shape[1]], dt, tag=f"w{i}", name=f"w{i}_sb"
                )
                for i in range(4)
            ]
            bti = (
                pp.tile([P, 11], f32, tag="bias", name="bias_sb")
                if add_bias
                else None
            )
            scr = (
                pp.tile([P, 2], f32, tag="scr", name="scr") if add_bias else None
            )
            acts = [
                [
                    pp.tile([P, B], dt, tag=f"a{li}_{i}", name=f"a{li}_{i}")
                    for i in range(n)
                ]
                for li, n in [(1, H1 // P), (2, H2 // P), (3, D1 // P)]
            ]
            acts.append([pp.tile([P, B], out_dt, tag="ot", name="ot")])

            # ---- PE warm-up continues inside the tile (no deps: raw
            # operands), keeping the HAM ramp fed while the first input
            # DMAs are in flight.
            for _ in range(warm):
                nc.tensor.matmul(wps[:], wlhs, wrhs, start=True, stop=True)

            # ---- input DMAs: x per batch-tile on the sync HWDGE queue,
            # weights on the scalar (ACT) HWDGE queue — two parallel
            # descriptor streams, each transfer issued in the order the
            # wavefront consumes it.  x is host-packed bt-major (col =
            # bt*2*BT + k*BT + b) so per-bt transfers are contiguous.
            kx = S // P

            def x_sl(bt, k, nk=1):
                return slice((bt * kx + k) * BT, (bt * kx + k + nk) * BT)

            # scalar queue: weights, first-needed-first.  (The gpsimd
            # SWDGE queue was tried for the critical first chunks and is
            # strictly worse: ~3us to first packet and ~100GB/s.)
            nc.scalar.dma_start(wts[0][:, 0:512], w_ds[0][:, 0:512])
            nc.scalar.dma_start(wts[0][:, 512:1024], w_ds[0][:, 512:1024])
            nc.scalar.dma_start(wts[1][:], w_ds[1][:])
            nc.scalar.dma_start(wts[3][:], w_ds[3][:])
            if add_bias:
                nc.scalar.dma_start(bti[:], b_d[:])
            # sync queue: x in per-k 128KB chunks in wavefront order, so
            # each chunk's completion sem fires the moment it lands; then
            # w3 (first needed at wavefront key 4).
            for bt in range(NBT):
                for k in range(kx):
                    sl = x_sl(bt, k)
                    nc.sync.dma_start(xt[:, sl], x_d[:, sl])
            nc.sync.dma_start(wts[2][:], w_ds[2][:])
            if add_bias:
                # advance ACT/DVE engine clocks past the bias DMA so the
                # real post-matmul ops carry a single (PE) wait each — the
                # AC/DVE instruction structs have one wait slot.
                nc.scalar.copy(scr[:, 0:1], bti[:, 0:1])
                nc.vector.tensor_copy(scr[:, 1:2], bti[:, 0:1])

            # ---- the 4-layer chain, emitted as a (bt + 2*layer) diagonal
            # wavefront: the PE's in-order queue then always has ready
            # later-layer work to chew while L1 waits on x DMAs.
            def x_rhs(k, bt):
                return xt[:, (bt * kx + k) * BT : (bt * kx + k + 1) * BT]

            sched = sorted(
                ((bt + 2 * li, -li, bt) for li in range(4) for bt in range(NBT))
            )
            for _, nli, bt in sched:
                li = -nli
                kc, mc, boff, relu = _LAYERS[li]
                wt = wts[li]
                dsts = acts[li]
                srcs = acts[li - 1] if li > 0 else None
                if li == 0:
                    # k-outer for every L1 batch-tile: each k sweep needs
                    # only one x chunk + half of w1 in SBUF, so the
                    # supply-paced phase runs with fine-grained waits
                    pss = [
                        psp.tile([P, BT], f32, tag="ps", name=f"ps_l0_{bt}_{m}")
                        for m in range(mc)
                    ]
                    for k in range(kc):
                        for m in range(mc):
                            nc.tensor.matmul(
                                pss[m][:],
                                wt[:, (k * mc + m) * P : (k * mc + m + 1) * P],
                                x_rhs(k, bt),
                                start=(k == 0),
                                stop=(k == kc - 1),
                            )
                else:
                    pss = None
                if li == 3 and bt == NBT - 1 and not add_bias:
                    # Last batch-tile of the last layer: column halves in
                    # two separate PSUM tiles (a shared tile would WAR-
                    # serialize half 1's first matmul behind half 0's
                    # PSUM->SBUF cast), each half cast and DMA'd on its
                    # own queue the moment it completes.  This shortens
                    # the post-matmul drain, which otherwise burns the
                    # HAM clock-gate hysteresis before the framework's
                    # semaphore teardown runs.
                    ot = acts[3][0]
                    h = BT // 2
                    o = bt * BT
                    for j in range(2):
                        ps = psp.tile([P, h], f32, tag="ps", name=f"ps_l3h{j}")
                        for k in range(kc):
                            nc.tensor.matmul(
                                ps[:],
                                wt[:, k * P : (k + 1) * P],
                                srcs[k][:, o + j * h : o + (j + 1) * h],
                                start=(k == 0),
                                stop=(k == kc - 1),
                            )
                        nc.vector.tensor_copy(
                            ot[:, o + j * h : o + (j + 1) * h], ps[:]
                        )
                        eng = nc.sync if j == 0 else nc.scalar
                        eng.dma_start(
                            out_d[:, o + j * h : o + (j + 1) * h],
                            ot[:, o + j * h : o + (j + 1) * h],
                        )
                    continue
                if li == 2 and bt == NBT - 1 and not add_bias:
                    # L3's last batch-tile feeds the terminal L4 chain,
                    # and L4-bt3's half-0 matmuls wait on these relus
                    # while the PE has nothing else queued.  Emit the
                    # relus in column halves, all half-0s first, so L4's
                    # first half starts ~0.6us earlier.
                    lps = []
                    for m in range(mc):
                        ps = psp.tile([P, BT], f32, tag="ps", name="ps_l2l")
                        for k in range(kc):
                            nc.tensor.matmul(
                                ps[:],
                                wt[:, (k * mc + m) * P : (k * mc + m + 1) * P],
                                srcs[k][:, bt * BT : (bt + 1) * BT],
                                start=(k == 0),
                                stop=(k == kc - 1),
                            )
                        lps.append(ps)
                    h = BT // 2
                    for half in range(2):
                        for m in range(mc):
                            dst = dsts[m][
                                :, bt * BT + half * h : bt * BT + (half + 1) * h
                            ]
                            src = lps[m][:, half * h : (half + 1) * h]
                            if m < mc // 2:
                                nc.scalar.activation(
                                    dst, src, mybir.ActivationFunctionType.Relu
                                )
                            else:
                                nc.vector.tensor_scalar_max(dst, src, 0.0)
                    continue
                for m in range(mc):
                    # fixed engine per dst tile: one writer per tile
                    use_act = (li < 3) and (m < mc // 2 or mc == 1)
                    if pss is not None:
                        ps = pss[m]
                    else:
                        ps = psp.tile([P, BT], f32, tag="ps", name="ps")
                        for k in range(kc):
                            rhs = (
                                x_rhs(k, bt)
                                if li == 0
                                else srcs[k][:, bt * BT : (bt + 1) * BT]
                            )
                            nc.tensor.matmul(
                                ps[:],
                                wt[:, (k * mc + m) * P : (k * mc + m + 1) * P],
                                rhs,
                                start=(k == 0),
                                stop=(k == kc - 1),
                            )
                    dst = dsts[m][:, bt * BT : (bt + 1) * BT]
                    if add_bias:
                        bias_ap = bti[:, boff + m : boff + m + 1]
                        if use_act:
                            func = (
                                mybir.ActivationFunctionType.Relu
                                if relu
                                else mybir.ActivationFunctionType.Identity
                            )
                            nc.scalar.activation(
                                dst, ps[:], func, bias=bias_ap
                            )
                        elif relu:
                            nc.vector.tensor_scalar(
                                dst,
                                ps[:],
                                bias_ap,
                                0.0,
                                mybir.AluOpType.add,
                                mybir.AluOpType.max,
                            )
                        else:
                            nc.vector.tensor_scalar_add(dst, ps[:], bias_ap)
                    elif use_act:
                        func = (
                            mybir.ActivationFunctionType.Relu
                            if relu
                            else mybir.ActivationFunctionType.Copy
                        )
                        nc.scalar.activation(dst, ps[:], func)
                    elif relu:
                        nc.vector.tensor_scalar_max(dst, ps[:], 0.0)
                    elif li == 3 and bt == NBT - 1:
                        # quarter the last copy so the final out-DMA
                        # chunks are small and start early
                        q = BT // 4
                        for j in range(4):
                            nc.vector.tensor_copy(
                                dst[:, j * q : (j + 1) * q],
                                ps[:, j * q : (j + 1) * q],
                            )
                    else:
                        nc.vector.tensor_copy(dst, ps[:])
                if li == 3:
                    ot = acts[3][0]
                    if bt < NBT - 1:
                        eng = nc.sync if bt % 2 == 0 else nc.scalar
                        eng.dma_start(
                            out_d[:, bt * BT : (bt + 1) * BT],
                            ot[:, bt * BT : (bt + 1) * BT],
                        )
                    else:
                        # last tile: quarter across both queues to
                        # shorten the final drain
                        q = BT // 4
                        o = bt * BT
                        for j in range(4):
                            eng = nc.sync if j % 2 == 0 else nc.scalar
                            eng.dma_start(
                                out_d[:, o + j * q : o + (j + 1) * q],
                                ot[:, o + j * q : o + (j + 1) * q],
                            )

            # ---- PE tail-pad: junk matmuls (no deps beyond program
            # order on the PE queue) issued after the last real matmul.
            # They keep the PE busy while the final activations/out-DMAs
            # drain, so the HAM clock-gate hysteresis window starts as
            # late as possible.
            for _ in range(junk):
                nc.tensor.matmul(wps[:], wlhs, wrhs, start=True, stop=True)

    # Post-tile clock-hold: runs after the tile-exit barrier, concurrent
    # with the start of the framework's teardown on the other queues.
    for _ in range(jpost):
        nc.tensor.matmul(wps[:], wlhs, wrhs, start=True, stop=True)
    _raw.close()
    nc.compile()
    return nc


_BUILT: dict[tuple, bass.Bass] = {}


def _cfg():
    dt_name = os.environ.get("MADPS_DT", "bf16")
    warm = int(os.environ.get("MADPS_WARM", "7"))
    junk = int(os.environ.get("MADPS_JUNK", "6"))
    pre = int(os.environ.get("MADPS_PRE", "1"))
    jpost = int(os.environ.get("MADPS_JPOST", "0"))
    return dt_name, warm, junk, pre, jpost


def _get_nc(
    dt_name: str,
    add_bias: bool,
    warm: int,
    junk: int,
    pre: int = 1,
    jpost: int = 4,
) -> bass.Bass:
    key = (dt_name, add_bias, warm, junk, pre, jpost)
    if key not in _BUILT:
        _BUILT[key] = _build(dt_name, add_bias, warm, junk, pre, jpost)
    return _BUILT[key]


def _np_dt(dt_name: str):
    if dt_name == "bf16":
        import ml_dtypes

        return ml_dtypes.bfloat16
    return np.float32


def _packw(w: np.ndarray, np_dt) -> np.ndarray:
    """[K, M] -> [128, (K/128)*M], k-chunk-major: col (k*mc + m)*128 + j."""
    k, m = w.shape
    kc = k // P
    return np.ascontiguousarray(
        w.reshape(kc, P, m).transpose(1, 0, 2).reshape(P, -1).astype(np_dt)
    )


def _prepare(inputs, dt_name):
    """Returns (add_bias, in_maps) for run_bass_kernel_spmd."""
    np_dt = _np_dt(dt_name)

    x = np.asarray(inputs["inputs"], dtype=np.float32)
    sel_s = np.asarray(inputs["laac_shallow"]).reshape(-1).astype(np.int64)
    sel_d = np.asarray(inputs["laac_deep"]).reshape(-1).astype(np.int64)
    Ws1 = np.asarray(inputs["Ws1"], dtype=np.float32)
    Ws2 = np.asarray(inputs["Ws2"], dtype=np.float32)
    Wd1 = np.asarray(inputs["Wd1"], dtype=np.float32)
    Wd2 = np.asarray(inputs["Wd2"], dtype=np.float32)
    bs1 = np.asarray(inputs["bs1"], dtype=np.float32)
    bs2 = np.asarray(inputs["bs2"], dtype=np.float32)
    bd1 = np.asarray(inputs["bd1"], dtype=np.float32)
    bd2 = np.asarray(inputs["bd2"], dtype=np.float32)

    add_bias = any(
        float(np.abs(b).max()) != 0.0 for b in (bs1, bs2, bd1, bd2)
    )

    in_maps = []
    for a in range(A):
        es, ed = int(sel_s[a]), int(sel_d[a])
        # bt-major packing: col = bt*(S//P)*BT + k*BT + b
        xp = np.ascontiguousarray(
            x[a]
            .reshape(NBT, BT, S // P, P)
            .transpose(3, 0, 2, 1)
            .reshape(P, -1)
            .astype(np_dt)
        )
        m = {
            "x": xp,
            "w1": _packw(Ws1[es], np_dt),
            "w2": _packw(Ws2[es], np_dt),
            "w3": _packw(Wd1[ed], np_dt),
            "w4": _packw(Wd2[ed], np_dt),
        }
        if add_bias:
            bias_cols = np.concatenate([bs1[es], bs2[es], bd1[ed], bd2[ed]])
            m["bias"] = np.ascontiguousarray(
                bias_cols.reshape(11, P).T, dtype=np.float32
            )
        in_maps.append(m)
    return add_bias, in_maps


def kernel(**inputs) -> np.ndarray:
    cfg = _cfg()
    dt_name = cfg[0]
    add_bias, in_maps = _prepare(inputs, dt_name)
    nc = _get_nc(dt_name, add_bias, *cfg[1:])
    res = run_bass_kernel_spmd(nc, in_maps, list(range(A)))
    out = np.stack(
        [np.asarray(res.results[a]["out"]).astype(np.float32).T for a in range(A)]
    )
    return np.ascontiguousarray(out)


# revision 50
# speedup vs baseline: 1.0486x; 1.0486x over previous
"""MADPSNet MoE-routing kernel for 8 Trainium2 NeuronCores.

The reference computes every expert on the full stacked input and then
gathers one expert per agent.  The routing indices (laac_shallow /
laac_deep) are host-visible numpy values, so we do the routing on the
host: per agent we select the 4 weight matrices of its chosen experts
and run only the selected chain

    x[2048,256] @ W1[256,512] -> relu -> @ W2[512,256] -> relu
                -> @ W3[256,512] -> relu -> @ W4[512,128] (+bias)

One agent per NeuronCore (A == 8 == n_cores), no collectives.

Layout: everything feature-major on chip (features on the 128
partitions, batch on the free dim).  The host pre-packs

    x   [128, 4096]     col = bt*1024 + k*512 + b  (bt-major batch tiles)
    wN  [128, K/128*M]  col = (k*mc + m)*128 + j   (k-chunk-major)
    bias[128, 11]       col j = 128-chunk j of [b1(4) b2(2) b3(4) b4(1)]

All tensors stream as bf16 (accumulate fp32 in PSUM; the 2e-2 rel-err
budget leaves ~5x headroom), which halves HBM traffic and lets
LDWEIGHTS use FWL.  Every DMA is a large contiguous transfer, issued
in compute-need order on the two HWDGE queues (x on sync/SP, weights
on scalar/ACT).  Matmuls accumulate fp32 in PSUM, bias+relu runs split
across ScalarE and VectorE with a fixed engine per destination tile,
and the layers are emitted as a (bt + 2*layer) diagonal wavefront so
the in-order PE queue always has ready work while L1 waits on x DMAs.
A few warm-up matmuls on an uninitialized scratch tile (no deps, so
they issue the moment the PE queue opens) keep the PE busy from kernel
start so the HAM clock un-throttles (1.2 -> 2.4 GHz) before the real
work arrives.  The kernel returns out^T [128, 2048] bf16 per core; the
host transposes back and upcasts.
"""

import os

import numpy as np

import concourse.bass as bass
import concourse.mybir as mybir
from concourse import bacc
from concourse.bass_utils import run_bass_kernel_spmd
from concourse.tile import TileContext

A, B, S = 8, 2048, 256
H1, H2, D1, D2 = 512, 256, 512, 128
P = 128
BT = 512            # batch tile (psum bank: 512 fp32)
NBT = B // BT

_DT_MAP = {
    "f32": mybir.dt.float32,
    "f32r": mybir.dt.float32r,
    "bf16": mybir.dt.bfloat16,
}

# layer: (k_chunks, m_chunks, bias col offset, relu?)
_LAYERS = [
    (S // P, H1 // P, 0, True),    # L1: 256 -> 512
    (H1 // P, H2 // P, 4, True),   # L2: 512 -> 256
    (H2 // P, D1 // P, 6, True),   # L3: 256 -> 512
    (D1 // P, D2 // P, 10, False), # L4: 512 -> 128
]


def _build(
    dt_name: str,
    add_bias: bool,
    warm: int,
    junk: int,
    pre: int = 1,
    jpost: int = 4,
) -> bass.Bass:
    dt = _DT_MAP[dt_name]
    f32 = mybir.dt.float32
    nc = bacc.Bacc(None, target_bir_lowering=False, debug=False)

    x_d = nc.dram_tensor("x", [P, (S // P) * B], dt, kind="ExternalInput")
    w_ds = [
        nc.dram_tensor("w1", [P, (S // P) * H1], dt, kind="ExternalInput"),
        nc.dram_tensor("w2", [P, (H1 // P) * H2], dt, kind="ExternalInput"),
        nc.dram_tensor("w3", [P, (H2 // P) * D1], dt, kind="ExternalInput"),
        nc.dram_tensor("w4", [P, (D1 // P) * D2], dt, kind="ExternalInput"),
    ]
    b_d = (
        nc.dram_tensor("bias", [P, 11], f32, kind="ExternalInput")
        if add_bias
        else None
    )
    out_dt = dt if dt == mybir.dt.bfloat16 else f32
    out_d = nc.dram_tensor("out", [D2, B], out_dt, kind="ExternalOutput")

    # Raw (non-tile) scratch for warm-up / clock-hold matmuls: SBUF
    # contents are garbage (never read downstream; the PSUM sink is
    # never read at all), so no memset and no dependencies anywhere.
    # Raw tensors let these matmuls run OUTSIDE the TileContext:
    #  - `pre` matmuls before the tile entry barrier start the HAM clock
    #    ramp ~1.1us earlier (the PE reaches the barrier last, delaying
    #    the input DMAs ~0.2us -- net win for one matmul);
    #  - `jpost` matmuls after the tile exit keep the PE busy into the
    #    framework's semaphore-zeroing teardown so it runs at full clock
    #    instead of the 4/8 idle-gated rate.
    import contextlib

    _raw = contextlib.ExitStack()
    wdt = f32 if dt == mybir.dt.float32r else dt
    wsb = _raw.enter_context(nc.sbuf_tensor([P, BT], wdt))
    wps = _raw.enter_context(nc.psum_tensor([P, BT], f32))
    wlhs = wsb[:, 0:P]
    wrhs = wsb[:]
    if dt == mybir.dt.float32r:
        wlhs = wlhs.bitcast(dt)
        wrhs = wrhs.bitcast(dt)
    for _ in range(pre):
        nc.tensor.matmul(wps[:], wlhs, wrhs, start=True, stop=True)

    with TileContext(nc) as tc:
        with (
            tc.tile_pool(name="persist", bufs=1) as pp,
            tc.tile_pool(name="psum", bufs=7, space="PSUM") as psp,
        ):
            xt = pp.tile([P, (S // P) * B], dt, tag="xt", name="xt")
            wts = [
                pp.tile(
                    [P, w_ds[i].shape[1]], dt, tag=f"w{i}", name=f"w{i}_sb"
                )
                for i in range(4)
            ]
            bti = (
                pp.tile([P, 11], f32, tag="bias", name="bias_sb")
                if add_bias
                else None
            )
            scr = (
                pp.tile([P, 2], f32, tag="scr", name="scr") if add_bias else None
            )
            acts = [
                [
                    pp.tile([P, B], dt, tag=f"a{li}_{i}", name=f"a{li}_{i}")
                    for i in range(n)
                ]
                for li, n in [(1, H1 // P), (2, H2 // P), (3, D1 // P)]
            ]
            acts.append([pp.tile([P, B], out_dt, tag="ot", name="ot")])

            # ---- PE warm-up continues inside the tile (no deps: raw
            # operands), keeping the HAM ramp fed while the first input
            # DMAs are in flight.
            for _ in range(warm):
                nc.tensor.matmul(wps[:], wlhs, wrhs, start=True, stop=True)

            # ---- input DMAs: x per batch-tile on the sync HWDGE queue,
            # weights on the scalar (ACT) HWDGE queue — two parallel
            # descriptor streams, each transfer issued in the order the
            # wavefront consumes it.  x is host-packed bt-major (col =
            # bt*2*BT + k*BT + b) so per-bt transfers are contiguous.
            kx = S // P

            def x_sl(bt, k, nk=1):
                return slice((bt * kx + k) * BT, (bt * kx + k + nk) * BT)

            # scalar queue: weights, first-needed-first.  (The gpsimd
            # SWDGE queue was tried for the critical first chunks and is
            # strictly worse: ~3us to first packet and ~100GB/s.)
            nc.scalar.dma_start(wts[0][:, 0:512], w_ds[0][:, 0:512])
            nc.scalar.dma_start(wts[0][:, 512:1024], w_ds[0][:, 512:1024])
            nc.scalar.dma_start(wts[1][:], w_ds[1][:])
            nc.scalar.dma_start(wts[3][:], w_ds[3][:])
            if add_bias:
                nc.scalar.dma_start(bti[:], b_d[:])
            # sync queue: x in per-k 128KB chunks in wavefront order, so
            # each chunk's completion sem fires the moment it lands; then
            # w3 (first needed at wavefront key 4).
            for bt in range(NBT):
                for k in range(kx):
                    sl = x_sl(bt, k)
                    nc.sync.dma_start(xt[:, sl], x_d[:, sl])
            nc.sync.dma_start(wts[2][:], w_ds[2][:])
            if add_bias:
                # advance ACT/DVE engine clocks past the bias DMA so the
                # real post-matmul ops carry a single (PE) wait each — the
                # AC/DVE instruction structs have one wait slot.
                nc.scalar.copy(scr[:, 0:1], bti[:, 0:1])
                nc.vector.tensor_copy(scr[:, 1:2], bti[:, 0:1])

            # ---- the 4-layer chain, emitted as a (bt + 2*layer) diagonal
            # wavefront: the PE's in-order queue then always has ready
            # later-layer work to chew while L1 waits on x DMAs.
            def x_rhs(k, bt):
                return xt[:, (bt * kx + k) * BT : (bt * kx + k + 1) * BT]

            sched = sorted(
                ((bt + 2 * li, -li, bt) for li in range(4) for bt in range(NBT))
            )
            for _, nli, bt in sched:
                li = -nli
                kc, mc, boff, relu = _LAYERS[li]
                wt = wts[li]
                dsts = acts[li]
                srcs = acts[li - 1] if li > 0 else None
                if li == 0:
                    # k-outer for every L1 batch-tile: each k sweep needs
                    # only one x chunk + half of w1 in SBUF, so the
                    # supply-paced phase runs with fine-grained waits
                    pss = [
                        psp.tile([P, BT], f32, tag="ps", name=f"ps_l0_{bt}_{m}")
                        for m in range(mc)
                    ]
                    for k in range(kc):
                        for m in range(mc):
                            nc.tensor.matmul(
                                pss[m][:],
                                wt[:, (k * mc + m) * P : (k * mc + m + 1) * P],
                                x_rhs(k, bt),
                                start=(k == 0),
                                stop=(k == kc - 1),
                            )
                else:
                    pss = None
                if li == 3 and bt == NBT - 1 and not add_bias:
                    # Last batch-tile of the last layer: column halves in
                    # two separate PSUM tiles (a shared tile would WAR-
                    # serialize half 1's first matmul behind half 0's
                    # PSUM->SBUF cast), each half cast and DMA'd on its
                    # own queue the moment it completes.  This shortens
                    # the post-matmul drain, which otherwise burns the
                    # HAM clock-gate hysteresis before the framework's
                    # semaphore teardown runs.
                    ot = acts[3][0]
                    h = BT // 2
                    o = bt * BT
                    for j in range(2):
                        ps = psp.tile([P, h], f32, tag="ps", name=f"ps_l3h{j}")
                        for k in range(kc):
                            nc.tensor.matmul(
                                ps[:],
                                wt[:, k * P : (k + 1) * P],
                                srcs[k][:, o + j * h : o + (j + 1) * h],
                                start=(k == 0),
                                stop=(k == kc - 1),
                            )
                        nc.vector.tensor_copy(
                            ot[:, o + j * h : o + (j + 1) * h], ps[:]
                        )
                        eng = nc.sync if j == 0 else nc.scalar
                        eng.dma_start(
                            out_d[:, o + j * h : o + (j + 1) * h],
                            ot[:, o + j * h : o + (j + 1) * h],
                        )
                    continue
                if li == 2 and bt == NBT - 1 and not add_bias:
                    # L3's last batch-tile feeds the terminal L4 chain,
                    # and L4-bt3's half-0 matmuls wait on these relus
                    # while the PE has nothing else queued.  Emit the
                    # relus in column halves, all half-0s first, so L4's
                    # first half starts ~0.6us earlier.
                    lps = []
                    for m in range(mc):
                        ps = psp.tile([P, BT], f32, tag="ps", name="ps_l2l")
                        for k in range(kc):
                            nc.tensor.matmul(
                                ps[:],
                                wt[:, (k * mc + m) * P : (k * mc + m + 1) * P],
                                srcs[k][:, bt * BT : (bt + 1) * BT],
                                start=(k == 0),
                                stop=(k == kc - 1),
                            )
                        lps.append(ps)
                    h = BT // 2
                    for half in range(2):
                        for m in range(mc):
                            dst = dsts[m][
                                :, bt * BT + half * h : bt * BT + (half + 1) * h
                            ]
                            src = lps[m][:, half * h : (half + 1) * h]
                            if m < mc // 2:
                                nc.scalar.activation(
                                    dst, src, mybir.ActivationFunctionType.Relu
                                )
                            else:
                                nc.vector.tensor_scalar_max(dst, src, 0.0)
                    continue
                for m in range(mc):
                    # fixed engine per dst tile: one writer per tile
                    use_act = (li < 3) and (m < mc // 2 or mc == 1)
                    if pss is not None:
                        ps = pss[m]
                    else:
                        ps = psp.tile([P, BT], f32, tag="ps", name="ps")
                        for k in range(kc):
                            rhs = (
                                x_rhs(k, bt)
                                if li == 0
                                else srcs[k][:, bt * BT : (bt + 1) * BT]
                            )
                            nc.tensor.matmul(
                                ps[:],
                                wt[:, (k * mc + m) * P : (k * mc + m + 1) * P],
                                rhs,
                                start=(k == 0),
                                stop=(k == kc - 1),
                            )
                    dst = dsts[m][:, bt * BT : (bt + 1) * BT]
                    if add_bias:
                        bias_ap = bti[:, boff + m : boff + m + 1]
                        if use_act:
                            func = (
                                mybir.ActivationFunctionType.Relu
                                if relu
                                else mybir.ActivationFunctionType.Identity
                            )
                            nc.scalar.activation(
                                dst, ps[:], func, bias=bias_ap
                            )
                        elif relu:
                            nc.vector.tensor_scalar(
                                dst,
                                ps[:],
                                bias_ap,
                                0.0,
                                mybir.AluOpType.add,
                                mybir.AluOpType.max,
                            )
                        else:
                            nc.vector.tensor_scalar_add(dst, ps[:], bias_ap)
                    elif use_act:
                        func = (
                            mybir.ActivationFunctionType.Relu
                            if relu
                            else mybir.ActivationFunctionType.Copy
                        )
                        nc.scalar.activation(dst, ps[:], func)
                    elif relu:
                        nc.vector.tensor_scalar_max(dst, ps[:], 0.0)
                    elif li == 3 and bt == NBT - 1:
                        # quarter the last copy so the final out-DMA
                        # chunks are small and start early
                        q = BT // 4
                        for j in range(4):
                            nc.vector.tensor_copy(
                                dst[:, j * q : (j + 1) * q],
                                ps[:, j * q : (j + 1) * q],
                            )
                    else:
                        nc.vector.tensor_copy(dst, ps[:])
                if li == 3:
                    ot = acts[3][0]
                    if bt < NBT - 1:
                        eng = nc.sync if bt % 2 == 0 else nc.scalar
                        eng.dma_start(
                            out_d[:, bt * BT : (bt + 1) * BT],
                            ot[:, bt * BT : (bt + 1) * BT],
                        )
                    else:
                        # last tile: quarter across both queues to
                        # shorten the final drain
                        q = BT // 4
                        o = bt * BT
                        for j in range(4):
                            eng = nc.sync if j % 2 == 0 else nc.scalar
                            eng.dma_start(
                                out_d[:, o + j * q : o + (j + 1) * q],
                                ot[:, o + j * q : o + (j + 1) * q],
                            )

            # ---- PE tail-pad: junk matmuls (no deps beyond program
            # order on the PE queue) issued after the last real matmul.
            # They keep the PE busy while the final activations/out-DMAs
            # drain, so the HAM clock-gate hysteresis window starts as
            # late as possible.
            for _ in range(junk):
                nc.tensor.matmul(wps[:], wlhs, wrhs, start=True, stop=True)

    # Post-tile clock-hold: runs after the tile-exit barrier, concurrent
    # with the start of the framework's teardown on the other queues.
    for _ in range(jpost):
        nc.tensor.matmul(wps[:], wlhs, wrhs, start=True, stop=True)
    _raw.close()
    nc.compile()
    return nc


_BUILT: dict[tuple, bass.Bass] = {}


def _cfg():
    dt_name = os.environ.get("MADPS_DT", "bf16")
    warm = int(os.environ.get("MADPS_WARM", "6"))
    junk = int(os.environ.get("MADPS_JUNK", "6"))
    pre = int(os.environ.get("MADPS_PRE", "1"))
    jpost = int(os.environ.get("MADPS_JPOST", "0"))
    return dt_name, warm, junk, pre, jpost


def _get_nc(
    dt_name: str,
    add_bias: bool,
    warm: int,
    junk: int,
    pre: int = 1,
    jpost: int = 4,
) -> bass.Bass:
    key = (dt_name, add_bias, warm, junk, pre, jpost)
    if key not in _BUILT:
        _BUILT[key] = _build(dt_name, add_bias, warm, junk, pre, jpost)
    return _BUILT[key]


def _np_dt(dt_name: str):
    if dt_name == "bf16":
        import ml_dtypes

        return ml_dtypes.bfloat16
    return np.float32


def _packw(w: np.ndarray, np_dt) -> np.ndarray:
    """[K, M] -> [128, (K/128)*M], k-chunk-major: col (k*mc + m)*128 + j."""
    k, m = w.shape
    kc = k // P
    return np.ascontiguousarray(
        w.reshape(kc, P, m).transpose(1, 0, 2).reshape(P, -1).astype(np_dt)
    )


def _prepare(inputs, dt_name):
    """Returns (add_bias, in_maps) for run_bass_kernel_spmd."""
    np_dt = _np_dt(dt_name)

    x = np.asarray(inputs["inputs"], dtype=np.float32)
    sel_s = np.asarray(inputs["laac_shallow"]).reshape(-1).astype(np.int64)
    sel_d = np.asarray(inputs["laac_deep"]).reshape(-1).astype(np.int64)
    Ws1 = np.asarray(inputs["Ws1"], dtype=np.float32)
    Ws2 = np.asarray(inputs["Ws2"], dtype=np.float32)
    Wd1 = np.asarray(inputs["Wd1"], dtype=np.float32)
    Wd2 = np.asarray(inputs["Wd2"], dtype=np.float32)
    bs1 = np.asarray(inputs["bs1"], dtype=np.float32)
    bs2 = np.asarray(inputs["bs2"], dtype=np.float32)
    bd1 = np.asarray(inputs["bd1"], dtype=np.float32)
    bd2 = np.asarray(inputs["bd2"], dtype=np.float32)

    add_bias = any(
        float(np.abs(b).max()) != 0.0 for b in (bs1, bs2, bd1, bd2)
    )

    in_maps = []
    for a in range(A):
        es, ed = int(sel_s[a]), int(sel_d[a])
        # bt-major packing: col = bt*(S//P)*BT + k*BT + b
        xp = np.ascontiguousarray(
            x[a]
            .reshape(NBT, BT, S // P, P)
            .transpose(3, 0, 2, 1)
            .reshape(P, -1)
            .astype(np_dt)
        )
        m = {
            "x": xp,
            "w1": _packw(Ws1[es], np_dt),
            "w2": _packw(Ws2[es], np_dt),
            "w3": _packw(Wd1[ed], np_dt),
            "w4": _packw(Wd2[ed], np_dt),
        }
        if add_bias:
            bias_cols = np.concatenate([bs1[es], bs2[es], bd1[ed], bd2[ed]])
            m["bias"] = np.ascontiguousarray(
                bias_cols.reshape(11, P).T, dtype=np.float32
            )
        in_maps.append(m)
    return add_bias, in_maps


def kernel(**inputs) -> np.ndarray:
    cfg = _cfg()
    dt_name = cfg[0]
    add_bias, in_maps = _prepare(inputs, dt_name)
    nc = _get_nc(dt_name, add_bias, *cfg[1:])
    res = run_bass_kernel_spmd(nc, in_maps, list(range(A)))
    out = np.stack(
        [np.asarray(res.results[a]["out"]).astype(np.float32).T for a in range(A)]
    )
    return np.ascontiguousarray(out)
